# revision 1
# baseline (speedup 1.0000x reference)
"""Trainium2 Bass kernel for nn_Attention (dense_transformer).

Reference computation (per batch n of 4):
  qkv = W_qkv @ x + b          (384, 4096)   [x flattened to (256, 64*64)]
  raw C-order reinterpret of qkv flat buffer as (4096, 384) -> q|k|v (4096,128) each
  scores = q @ k.T / 64        (4096, 4096)
  soft = softmax(scores, axis=-2)             [column softmax]
  out = soft @ v               (4096, 128)
  raw reinterpret of out as (128, 4096)
  y = W_out @ out2 + b_out     (256, 4096)

Sharding: 8 cores = 4 batches x 2 column-chunks (j-axis of the score
matrix = rows of k/v). Column-softmax stats (over i) are local to a
j-chunk; each core produces a partial y, host sums the pair.

The SPMD graph is identical on all cores; the j-half selection is encoded
host-side by rotating the qkv output channels by 192 for odd cores (which
rotates the reinterpreted sequence axis by 2048) and rotating W_out's
e-axis by 64 to compensate on the output side.

Compute layout (per core):
  stage 1: F = W_qkv @ x + b as 3 o-tiles (128, 4096) bf16 -> DRAM fbuf,
           written as half-tiles fed by ACT(lo)/DVE(hi) bias-copies.
  loads:   qT (d,i) and kT (d,j) via xbar transpose-DMA from the (4096,384)
           reinterpret view of fbuf (one batched xbar window); v (j,d) plain
           via SWDGE. All split at 512-aligned boundaries per covering
           F o-tile so phase A starts before stage 1 fully drains.
  phase A (per j-block of 128): Pt[j,i] = exp(kT_jb.T q / 64), four
           (128,1024) exps with fused column-sum accum_out; Z -> 1/Z ->
           v scaled in place. The output matmuls for hw-groups 0-3
           accumulate inline in PSUM banks 4-7 (permuted i-axis
           i' = hb*128+e via a strided rhs AP on P, software-pipelined one
           j-block behind the stats), then drain through transpose/proj2.
  phase B+C+proj2 for groups 4-7, fused per 512-wide group: 16
           accumulate-MMs -> copy -> 4 TensorE transposes -> out2g ->
           proj2 MMs + bias -> y, per-half y DMAs; banks recycle via a
           bufs=2 pool. PSUM bank g == out2 group g throughout.
"""

import numpy as np
import ml_dtypes

import concourse.bass as bass
import concourse.bacc as bacc
import concourse.mybir as mybir
from concourse.bass_utils import run_bass_kernel_spmd
from concourse.tile import TileContext, add_dep_helper
from concourse.masks import make_identity

BF16 = mybir.dt.bfloat16
F32 = mybir.dt.float32
AF = mybir.ActivationFunctionType

N, C, E, O, HW = 4, 256, 128, 384, 4096
JC = HW // 2          # j-chunk per core
NJB = JC // 128       # 16 j-blocks
SCALE = 1.0 / 64.0    # 1/sqrt(HW)

_CACHE = {}


def build_nc():
    nc = bacc.Bacc("TRN2", target_bir_lowering=False, debug=False, num_devices=8)

    x_ext = nc.dram_tensor("x", [C, HW], BF16, kind="ExternalInput").ap()
    wqkvT_ext = nc.dram_tensor("wqkvT", [C, O], BF16, kind="ExternalInput").ap()
    bqkv_ext = nc.dram_tensor("bqkv", [O, 1], F32, kind="ExternalInput").ap()
    woutT_ext = nc.dram_tensor("woutT", [E, C], BF16, kind="ExternalInput").ap()
    bout_ext = nc.dram_tensor("bout", [C, 1], F32, kind="ExternalInput").ap()
    y_ext = nc.dram_tensor("out", [C, HW], BF16, kind="ExternalOutput").ap()

    fbuf = nc.dram_tensor("fbuf", [O * HW], BF16).ap()
    fview_o = fbuf.rearrange("(o hw) -> o hw", hw=HW)   # (384, 4096) write view
    fview_i = fbuf.rearrange("(i j) -> i j", j=O)        # (4096, 384) read view

    # persistent SBUF (fixed allocations; not subject to pool slot reuse).
    # qT/kT/v are split at 512-aligned boundaries covered by successive F
    # o-tiles so phase A can start before stage 1 fully drains.
    QSPL = [0, 1024, 2560, HW]       # parts covered by F o-tiles 0/1/2
    KSPL = [0, 1024, JC]             # parts covered by F o-tiles 0/1
    VSPL = [0, 1280, JC]
    qTp = [nc.alloc_sbuf_tensor(f"qT{i}", [128, QSPL[i + 1] - QSPL[i]], BF16).ap()
           for i in range(3)]
    kTp = [nc.alloc_sbuf_tensor(f"kT{i}", [128, KSPL[i + 1] - KSPL[i]], BF16).ap()
           for i in range(2)]
    vp = [nc.alloc_sbuf_tensor(f"v{i}", [128, VSPL[i + 1] - VSPL[i]], BF16).ap()
          for i in range(2)]

    def qT_sl(i0, w=512):
        p = 0 if i0 < 1024 else (1 if i0 < 2560 else 2)
        a = i0 - QSPL[p]
        assert a + w <= QSPL[p + 1] - QSPL[p]
        return qTp[p][:, a:a + w]

    def kT_sl(jb):
        p = 0 if jb < 8 else 1
        a = jb * 128 - KSPL[p]
        return kTp[p][:, a:a + 128]

    def v_sl(jb):
        p = 0 if jb < 10 else 1
        a = jb * 128 - VSPL[p]
        return vp[p][:, a:a + 128]

    zacc = nc.alloc_sbuf_tensor("zacc", [128, 64], F32).ap()
    zsum = nc.alloc_sbuf_tensor("zsum", [128, 16], F32).ap()
    zinv = nc.alloc_sbuf_tensor("zinv", [128, 16], F32).ap()
    outTg = [nc.alloc_sbuf_tensor(f"outTg{g}", [128, 512], BF16).ap()
             for g in range(8)]
    out2g = [nc.alloc_sbuf_tensor(f"out2g{g}", [128, 512], BF16).ap()
             for g in range(8)]
    P = nc.alloc_sbuf_tensor("P", [128, NJB * HW], BF16).ap()

    with TileContext(nc) as tc:
        with tc.tile_pool(name="consts", bufs=1) as consts:
            # ---- constants (bias first: it gates the first stage-1 copies) ----
            bias = consts.tile([128, 8], F32, name="bias", tag="bias")
            bq = [bias[:, i:i + 1] for i in range(3)]
            bo = [bias[:, 3 + i:4 + i] for i in range(2)]
            for ob in range(3):
                nc.scalar.dma_start(out=bq[ob], in_=bqkv_ext[ob * 128:(ob + 1) * 128, :])
            wq_all = consts.tile([128, 2 * O], BF16, name="wq_all", tag="wq_all")
            wqT = [wq_all[:, 0:O], wq_all[:, O:2 * O]]
            for cb in range(2):
                nc.scalar.dma_start(out=wqT[cb], in_=wqkvT_ext[cb * 128:(cb + 1) * 128, :])
            for cb in range(2):
                nc.scalar.dma_start(out=bo[cb], in_=bout_ext[cb * 128:(cb + 1) * 128, :])
            misc = consts.tile([128, C + 128], BF16, name="misc", tag="misc")
            woutT = misc[:, 0:C]
            ident = misc[:, C:C + 128]
            nc.scalar.dma_start(out=woutT, in_=woutT_ext[:])
            make_identity(nc, ident)
            scratch = consts.tile([128, 1], F32, name="scratch", tag="scratch")
            nc.vector.memset(scratch[:], 0.0)
            nc.scalar.activation(scratch[:], scratch[:], AF.Exp)

            # ---- PE warmup: dummy matmuls so HAM is at full clock before
            #      stage 1 (identity data; output never read) ----
            wsrc = consts.tile([128, 128], BF16, name="wsrc", tag="wsrc")
            nc.vector.memset(wsrc[:], 1.0)
            with tc.tile_pool(name="psW", bufs=1, space="PSUM") as psW:
                wtile = psW.tile([128, 128], F32, tag="warm")
                for _ in range(16):
                    nc.tensor.matmul(wtile[:], wsrc[:], wsrc[:], start=True, stop=True)

            # ---- x loads (2 x 1MB, sync ring) ----
            early = tc.alloc_tile_pool(name="early", bufs=1)
            # x split (cb, half) so the first matmuls start after 2 chunks
            xsb = [[early.tile([128, HW // 2], BF16, name=f"x{cb}{h}", tag=f"x{cb}{h}")
                    for h in range(2)] for cb in range(2)]
            Fsb = [[early.tile([128, HW // 2], BF16, name=f"F{i}{hh}", tag=f"F{i}{hh}")
                    for hh in range(2)] for i in range(3)]
            for h in range(2):
                for cb in range(2):
                    nc.sync.dma_start(
                        out=xsb[cb][h][:],
                        in_=x_ext[cb * 128:(cb + 1) * 128,
                                  h * (HW // 2):(h + 1) * (HW // 2)])

            # ---- stage 1: qkv projection -> Fsb o-tiles -> fbuf,
            #      with q/k/v part-loads woven in right after each F write ----
            with tc.tile_pool(name="psF", bufs=4, space="PSUM") as psF:
                f_writes = []
                vlds = []
                for ob in range(3):
                    for nch in range(8):
                        pf = psF.tile([128, 512], F32, tag="pf")
                        h, o512 = nch // 4, (nch % 4) * 512
                        sl = slice(nch * 512, (nch + 1) * 512)
                        nc.tensor.matmul(
                            pf[:], wqT[0][:, ob * 128:(ob + 1) * 128],
                            xsb[0][h][:, o512:o512 + 512],
                            start=True, stop=False,
                        )
                        nc.tensor.matmul(
                            pf[:], wqT[1][:, ob * 128:(ob + 1) * 128],
                            xsb[1][h][:, o512:o512 + 512],
                            start=False, stop=True,
                        )
                        fb = Fsb[ob][nch // 4]
                        fsl = slice((nch % 4) * 512, (nch % 4 + 1) * 512)
                        if nch < 4:
                            nc.vector.tensor_scalar_add(fb[:, fsl], pf[:], bq[ob])
                        else:
                            nc.scalar.activation(fb[:, fsl], pf[:], AF.Identity,
                                                 bias=bq[ob])
                    ws = []
                    for hh in range(2):
                        eng = nc.sync if hh == 0 else nc.scalar
                        ws.append(eng.dma_start(
                            out=fview_o[ob * 128:(ob + 1) * 128,
                                        hh * (HW // 2):(hh + 1) * (HW // 2)],
                            in_=Fsb[ob][hh][:],
                        ))
                    f_writes.append(ws)
                    # v loads (plain copies, scalar ring) right after their F write
                    if ob == 0:
                        r = nc.gpsimd.dma_start(
                            out=vp[0].rearrange("p (t d) -> p t d", d=128),
                            in_=fview_i[0:1280, 2 * E:3 * E].rearrange(
                                "(t p) d -> p t d", p=128))
                        vlds.append(r)
                        for w2 in ws:
                            add_dep_helper(r.ins, w2.ins, reason="fbuf RAW")
                    elif ob == 1:
                        r = nc.gpsimd.dma_start(
                            out=vp[1].rearrange("p (t d) -> p t d", d=128),
                            in_=fview_i[1280:2048, 2 * E:3 * E].rearrange(
                                "(t p) d -> p t d", p=128))
                        vlds.append(r)
                        for w2 in ws:
                            add_dep_helper(r.ins, w2.ins, reason="fbuf RAW")
                # all xbar transposes batched in one mode-window (sync ring).
                # The first three only order behind F0/F1 so they run before
                # F2's writes; v loads are pushed behind the transposes in the
                # bandwidth queue (v is not needed until the first stats).
                tr_specs = [
                    (qTp[0], fview_i[0:1024, 0:E], 0, 2),
                    (kTp[0], fview_i[0:1024, E:2 * E], 0, 2),
                    (qTp[1], fview_i[1024:2560, 0:E], 1, 2),
                    (qTp[2], fview_i[2560:HW, 0:E], 2, 3),
                    (kTp[1], fview_i[1024:2048, E:2 * E], 1, 3),
                ]
                trs = []
                for dst, srcap, dep, nhint in tr_specs:
                    rt = nc.sync.dma_start_transpose(out=dst[:], in_=srcap)
                    trs.append(rt)
                    for w2 in f_writes[dep]:
                        add_dep_helper(rt.ins, w2.ins, reason="fbuf RAW")
                    for ws2 in f_writes[:nhint]:
                        for w2 in ws2:
                            add_dep_helper(rt.ins, w2.ins, sync=False,
                                           reason="xbar window after copies")
            early.release()

            # ---- phase A: scores + exp(1024-wide, fused column sums),
            #      with groups 0-3 of the output matmul inlined (banks 4-7),
            #      software-pipelined one j-block behind the stats ----
            P3 = P.rearrange("p (jb e hb) -> p jb hb e", jb=NJB, hb=32)
            with tc.tile_pool(name="psBi", bufs=1, space="PSUM") as psBi:
                obi = [psBi.tile([128, 512], F32, name=f"obi{g}", tag=f"obi{g}")
                       for g in range(4)]

                def inline_mms(jb):
                    for g in range(4):
                        nc.tensor.matmul(
                            obi[g][:], v_sl(jb), P3[:, jb, 4 * g:4 * g + 4, :],
                            start=(jb == 0), stop=(jb == NJB - 1),
                        )

                with tc.tile_pool(name="psA", bufs=2, space="PSUM") as psA:
                    def score_exp(jb, h):
                        pa = psA.tile([128, 1024], F32, tag="pa")
                        for n2 in range(2):
                            i0 = h * 1024 + n2 * 512
                            nc.tensor.matmul(
                                pa[:, n2 * 512:(n2 + 1) * 512],
                                kT_sl(jb), qT_sl(i0),
                                start=True, stop=True,
                            )
                        nc.scalar.activation(
                            out=P[:, jb * HW + h * 1024: jb * HW + (h + 1) * 1024],
                            in_=pa[:],
                            func=AF.Exp,
                            scale=SCALE,
                            accum_out=zacc[:, jb * 4 + h: jb * 4 + h + 1],
                        )

                    # h0/h1 only touch qT parts 0-1; h2/h3 need part 2 which
                    # lands last. Lead with h0/h1 of the first three j-blocks
                    # so the exp chain stays dense while qT2 is in flight.
                    LEAD = 4
                    for jb in range(LEAD):
                        score_exp(jb, 0)
                    for jb in range(LEAD):
                        score_exp(jb, 1)
                    for jb in range(NJB):
                        score_exp(jb, 2)
                        score_exp(jb, 3)
                        nc.vector.reduce_sum(
                            out=zsum[:, jb:jb + 1],
                            in_=zacc[:, jb * 4:(jb + 1) * 4],
                            axis=mybir.AxisListType.X,
                        )
                        nc.vector.reciprocal(zinv[:, jb:jb + 1], zsum[:, jb:jb + 1])
                        nc.vector.tensor_scalar_mul(
                            v_sl(jb), v_sl(jb), zinv[:, jb:jb + 1],
                        )
                        if jb + LEAD < NJB:
                            score_exp(jb + LEAD, 0)
                            score_exp(jb + LEAD, 1)
                        if jb > 2:
                            inline_mms(jb - 3)
                    inline_mms(NJB - 3)
                    inline_mms(NJB - 2)
                    inline_mms(NJB - 1)

                # drain inline groups 0-3 through transpose/proj2 (C-part only)
                with tc.tile_pool(name="psC0", bufs=2, space="PSUM") as psC0, \
                     tc.tile_pool(name="psY0", bufs=2, space="PSUM") as psY0, \
                     tc.tile_pool(name="late0", bufs=1) as late0:
                    yg0 = [[late0.tile([128, 512], BF16, name=f"yg{cb}_{g}",
                                       tag=f"yg{cb}_{g}") for g in range(4)]
                           for cb in range(2)]
                    for g in range(4):
                        if g % 2 == 0:
                            nc.scalar.copy(outTg[g][:], obi[g][:])
                        else:
                            nc.vector.tensor_copy(outTg[g][:], obi[g][:])
                        tp = psC0.tile([128, 512], BF16, tag="tp0")
                        for s in range(4):
                            nc.tensor.transpose(
                                tp[:, s * 128:(s + 1) * 128],
                                outTg[g][:, s * 128:(s + 1) * 128],
                                ident,
                            )
                        if g % 2 == 0:
                            nc.vector.tensor_copy(out2g[g][:], tp[:])
                        else:
                            nc.scalar.copy(out2g[g][:], tp[:])
                        for cb in range(2):
                            py = psY0.tile([128, 512], F32, tag="py0")
                            nc.tensor.matmul(
                                py[:], woutT[:, cb * 128:(cb + 1) * 128], out2g[g][:],
                                start=True, stop=True,
                            )
                            dst = yg0[cb][g][:]
                            if cb == 0:
                                nc.scalar.activation(dst, py[:], AF.Identity,
                                                     bias=bo[cb])
                            else:
                                nc.vector.tensor_scalar_add(dst, py[:], bo[cb])
                            [nc.sync, nc.scalar][cb].dma_start(
                                out=y_ext[cb * 128:(cb + 1) * 128,
                                          g * 512:(g + 1) * 512],
                                in_=dst)

            # ---- phase B + C + proj2, fused per 512-wide group ----
            # outT is produced with permuted i-axis: i' = hb*128 + e (hb = hw
            # block, e = embed row), so PSUM bank g holds exactly the data for
            # out2 group g: transpose outT'[:, hb*128:+128].T = out2[:, hb*128:+128].
            # The permutation comes free via a strided rhs AP on P.
            with tc.tile_pool(name="psB", bufs=2, space="PSUM") as psB, \
                 tc.tile_pool(name="psC", bufs=2, space="PSUM") as psC, \
                 tc.tile_pool(name="psY", bufs=3, space="PSUM") as psY, \
                 tc.tile_pool(name="late", bufs=1) as late:
                yg1 = [[late.tile([128, 512], BF16, name=f"yb{cb}_{g}",
                                  tag=f"yb{cb}_{g}") for g in range(4)]
                       for cb in range(2)]
                for g in range(4, 8):
                    ob_ps = psB.tile([128, 512], F32, tag="ob_ps")
                    for jb in range(NJB):
                        nc.tensor.matmul(
                            ob_ps[:],
                            v_sl(jb),
                            P3[:, jb, 4 * g:4 * g + 4, :],
                            start=(jb == 0), stop=(jb == NJB - 1),
                        )
                    if g % 2 == 0:
                        nc.scalar.copy(outTg[g][:], ob_ps[:])
                    else:
                        nc.vector.tensor_copy(outTg[g][:], ob_ps[:])
                    tp = psC.tile([128, 512], BF16, tag="tp")
                    for s in range(4):
                        nc.tensor.transpose(
                            tp[:, s * 128:(s + 1) * 128],
                            outTg[g][:, s * 128:(s + 1) * 128],
                            ident,
                        )
                    if g % 2 == 0:
                        nc.vector.tensor_copy(out2g[g][:], tp[:])
                    else:
                        nc.scalar.copy(out2g[g][:], tp[:])
                    for cb in range(2):
                        py = psY.tile([128, 512], F32, tag="py")
                        nc.tensor.matmul(
                            py[:], woutT[:, cb * 128:(cb + 1) * 128], out2g[g][:],
                            start=True, stop=True,
                        )
                        dst = yg1[cb][g - 4][:]
                        if cb == 0:
                            nc.scalar.activation(dst, py[:], AF.Identity, bias=bo[cb])
                        else:
                            nc.vector.tensor_scalar_add(dst, py[:], bo[cb])
                        [nc.sync, nc.scalar][cb].dma_start(
                            out=y_ext[cb * 128:(cb + 1) * 128,
                                      g * 512:(g + 1) * 512],
                            in_=dst)

    nc.compile()
    return nc


def get_nc():
    if "nc" not in _CACHE:
        _CACHE["nc"] = build_nc()
    return _CACHE["nc"]


def make_in_maps(x, W_qkv, b_qkv, W_out, b_out):
    x = np.asarray(x, dtype=np.float32)
    W_qkv = np.asarray(W_qkv, dtype=np.float32)
    b_qkv = np.asarray(b_qkv, dtype=np.float32)
    W_out = np.asarray(W_out, dtype=np.float32)
    b_out = np.asarray(b_out, dtype=np.float32)

    operm = (np.arange(O) + O // 2) % O      # rotate qkv channels by 192
    eperm = (np.arange(E) + E // 2) % E      # rotate e-axis by 64

    halves = []
    for h in range(2):
        if h == 0:
            wq, bqv, wo, bov = W_qkv, b_qkv, W_out, b_out
        else:
            wq = W_qkv[operm]
            bqv = b_qkv[operm]
            wo = W_out[:, eperm]
            bov = np.zeros_like(b_out)
        halves.append({
            "wqkvT": np.ascontiguousarray(wq.T).astype(ml_dtypes.bfloat16),
            "bqkv": np.ascontiguousarray(bqv.reshape(O, 1)),
            "woutT": np.ascontiguousarray(wo.T).astype(ml_dtypes.bfloat16),
            "bout": np.ascontiguousarray(bov.reshape(C, 1)),
        })

    xb = [np.ascontiguousarray(x[n].reshape(C, HW)).astype(ml_dtypes.bfloat16)
          for n in range(N)]
    in_maps = []
    for core in range(8):
        n, h = core // 2, core % 2
        m = {"x": xb[n]}
        m.update(halves[h])
        in_maps.append(m)
    return in_maps


def run(inputs, trace=False, **kw):
    nc = get_nc()
    in_maps = make_in_maps(**inputs)
    res = run_bass_kernel_spmd(nc, in_maps, core_ids=list(range(8)), trace=trace, **kw)
    ys = [np.asarray(res.results[i]["out"], dtype=np.float32) for i in range(8)]
    y = np.stack([ys[2 * n] + ys[2 * n + 1] for n in range(N)])
    return y.reshape(N, C, 64, 64), res


def kernel(**inputs):
    y, _ = run(inputs, trace=False)
    return y



# revision 29
# speedup vs baseline: 1.2157x; 1.2157x over previous
"""Trainium2 Bass kernel for nn_Attention (dense_transformer), v2.

Reference computation (per batch n of 4):
  qkv = W_qkv @ x + b          (384, 4096)   [x flattened to (256, 64*64)]
  raw C-order reinterpret of qkv flat buffer as (4096, 384) -> q|k|v (4096,128)
  scores = q @ k.T / 64        (4096, 4096)
  soft = softmax(scores, axis=-2)             [column softmax]
  out = soft @ v               (4096, 128)
  raw reinterpret of out as (128, 4096)
  y = W_out @ out2 + b_out     (256, 4096)

Sharding: 8 cores = 4 batches x 2 j-halves (t-halves of the permuted j
enumeration; the host-side 192-rotation of qkv channels and 64-rotation of
W_out's e-axis make the SPMD program identical on all cores). Host sums the
per-pair partial y.

Dataflow (all on-chip; no DRAM roundtrip):
  The raw reinterpret has block structure: with flat = 128*u + r,
  u = 3*tok + (qkv sel) = 32*o + sc, the d-axis of q/k/v equals r = s%128.
  Stage 1 computes, per 128-col s-block sc, FT = x_blk^T @ W_var[sc%3] in
  PSUM, where the host pre-permutes W's output channels per residue so each
  block lands as contiguous [q(128) | k-half(64) | v-half(64)] (the other
  t-half belongs to the paired core and is never computed). One 256-wide
  copy per block drains PSUM->SBUF: ACT plain copies for sc<4 (bias added
  in PSUM via a ones-row matmul against a bias-row input), DVE
  tensor-tensor adds against a host-broadcast bias tile for the rest.
  kT is restaged contiguously (4 wide DVE copies); v is untransposed with
  2x64-wide TensorE transposes (tile_position for the upper half).

  Phase A: per (jb, h-quarter): 2 score MMs (128j x 1024i) -> PSUM(2 banks,
  double buffered), ACT exp (1024-wide) -> P bf16 (persistent SBUF).
  Z per chunk: ACT accum_out on h1/h3 (h3 gates the per-jb stats), DVE
  reduces for h0/h2. Per-jb stats: Z sum, reciprocal, v-block scale (DVE).
  Output accumulation (8 groups of 512 i' = sc-quads) rotates 4 PSUM banks:
  groups 0-3 chain jb0-7 inline, spill to SBUF f32, groups 4-7 then chain
  jb0-15 (burst jb0-7 from persistent P + live jb8-15); groups 0-3 finish
  jb8-15 in the tail on psA's freed banks, their drain adding the spill.
  All non-score PE work is emitted via a lag-2 side queue (<=4 per chunk)
  so nothing with unmet deps ever blocks the in-order PE stream ahead of
  the next score MMs.

  Tail: per group: PSUM -> outTa bf16, 4 TensorE transposes -> out2a in
  accumulator (sc) order. The psi_q permutation folds into proj2's rhs
  access patterns (stride-3 chunk gathers; y-groups 2 and 5 split into two
  accumulating MMs). y = woutT-mm + bias (ACT/DVE), DMA per (group, chalf).
"""

import numpy as np
import ml_dtypes

import concourse.bass as bass
import concourse.bacc as bacc
import concourse.mybir as mybir
from concourse.bass_utils import run_bass_kernel_spmd
from concourse.tile import TileContext, add_dep_helper
from concourse.masks import make_identity
from concourse.alu_op_type import AluOpType

BF16 = mybir.dt.bfloat16
F32 = mybir.dt.float32
AF = mybir.ActivationFunctionType
AX = mybir.AxisListType

N, C, E, O, HW = 4, 256, 128, 384, 4096
JC = HW // 2          # j per core
NJB = JC // 128       # 16 j-blocks
SCALE = 1.0 / 64.0    # 1/sqrt(HW)

_CACHE = {}


def _psiq_inv(m):
    if m <= 10:
        return 3 * m
    if m <= 21:
        return 3 * (m - 11) + 1
    return 3 * (m - 22) + 2


def _proj2_runs(G):
    """Maximal stride-3 source-chunk runs feeding y columns [4G*128,(4G+4)*128)."""
    srcs = [_psiq_inv(4 * G + k) for k in range(4)]
    runs = []
    for s in srcs:
        if runs and s == runs[-1][-1] + 3:
            runs[-1].append(s)
        else:
            runs.append([s])
    return runs


def build_nc():
    nc = bacc.Bacc("TRN2", target_bir_lowering=False, debug=False, num_devices=8)

    x_ext = nc.dram_tensor("x", [C, HW], BF16, kind="ExternalInput").ap()
    wqkvT_ext = nc.dram_tensor("wqkvT", [C, 768], BF16, kind="ExternalInput").ap()
    btile_ext = nc.dram_tensor("btile", [128, 768], BF16, kind="ExternalInput").ap()
    brow_ext = nc.dram_tensor("brow", [1, 768], BF16, kind="ExternalInput").ap()
    woutT_ext = nc.dram_tensor("woutT", [E, C], BF16, kind="ExternalInput").ap()
    bout_ext = nc.dram_tensor("bout", [C, 1], F32, kind="ExternalInput").ap()
    y_ext = nc.dram_tensor("out", [C, HW], BF16, kind="ExternalOutput").ap()

    # persistent SBUF
    xsb = [nc.alloc_sbuf_tensor(f"x{cb}", [128, HW], BF16).ap() for cb in range(2)]
    QKV = nc.alloc_sbuf_tensor("QKV", [128, 32 * 256], BF16).ap()
    kT = nc.alloc_sbuf_tensor("kT", [128, JC], BF16).ap()
    vsb = nc.alloc_sbuf_tensor("vsb", [128, JC], BF16).ap()
    P = nc.alloc_sbuf_tensor("P", [128, NJB * HW], BF16).ap()
    outTa = nc.alloc_sbuf_tensor("outTa", [128, HW], BF16).ap()
    out2a = nc.alloc_sbuf_tensor("out2a", [128, HW], BF16).ap()
    spill = [nc.alloc_sbuf_tensor(f"spill{g}", [128, 512], F32).ap() for g in range(4)]
    zacc = nc.alloc_sbuf_tensor("zacc", [128, 64], F32).ap()
    zsum = nc.alloc_sbuf_tensor("zsum", [128, 16], F32).ap()
    zinv = nc.alloc_sbuf_tensor("zinv", [128, 16], F32).ap()

    def v_sl(jb):
        return vsb[:, jb * 128:(jb + 1) * 128]

    with TileContext(nc) as tc:
        with tc.tile_pool(name="consts", bufs=1) as consts:
            # ---- weights/constants ----
            brow = consts.tile([1, 768], BF16, name="brow", tag="brow")
            nc.sync.dma_start(out=brow, in_=brow_ext[:])
            wq_all = consts.tile([128, 2 * 768], BF16, name="wq_all", tag="wq_all")
            for cb in range(2):
                nc.scalar.dma_start(out=wq_all[:, cb * 768:(cb + 1) * 768],
                                    in_=wqkvT_ext[cb * 128:(cb + 1) * 128, :])
            btile = consts.tile([128, 768], BF16, name="btile", tag="btile")
            nc.gpsimd.dma_start(out=btile, in_=btile_ext[:])
            ones1 = consts.tile([1, 128], BF16, name="ones1", tag="ones1")
            nc.vector.memset(ones1[:], 1.0)

            def wq_sl(cb, r):
                return wq_all[:, cb * 768 + r * 256: cb * 768 + (r + 1) * 256]

            misc = consts.tile([128, C + 128], BF16, name="misc", tag="misc")
            woutT = misc[:, 0:C]
            ident = misc[:, C:C + 128]
            nc.gpsimd.dma_start(out=woutT, in_=woutT_ext[:])
            make_identity(nc, ident)
            bo2 = consts.tile([128, 2], F32, name="bo2", tag="bo2")
            bo = [bo2[:, cb:cb + 1] for cb in range(2)]
            for cb in range(2):
                nc.gpsimd.dma_start(out=bo[cb], in_=bout_ext[cb * 128:(cb + 1) * 128, :])
            # Exp table preload
            scratch = consts.tile([128, 1], F32, name="scratch", tag="scratch")
            nc.vector.memset(scratch[:], 0.0)
            nc.scalar.activation(scratch[:], scratch[:], AF.Exp)

            # ---- PE warmup (p-state ramp) ----
            wsrc = consts.tile([128, 128], BF16, name="wsrc", tag="wsrc")
            nc.vector.memset(wsrc[:], 1.0)
            with tc.tile_pool(name="psW", bufs=1, space="PSUM") as psW:
                wtile = psW.tile([128, 128], F32, tag="warm")
                for _ in range(16):
                    nc.tensor.matmul(wtile[:], wsrc[:], wsrc[:], start=True, stop=True)

            # ---- x loads: 8 quarter-chunks on sync/scalar rings ----
            for qtr in range(4):
                for cb in range(2):
                    eng = nc.sync if (qtr + cb) % 2 == 0 else nc.scalar
                    eng.dma_start(
                        out=xsb[cb][:, qtr * 1024:(qtr + 1) * 1024],
                        in_=x_ext[cb * 128:(cb + 1) * 128,
                                  qtr * 1024:(qtr + 1) * 1024])

            # ---- stage 1: FT blocks (variant-permuted W) -> QKV + v transposes ----
            QKVv = QKV.rearrange("p (b c) -> p b c", c=256)

            def v_src(b):
                return QKVv[:, b, 192:256]

            with tc.tile_pool(name="psF", bufs=3, space="PSUM") as psF, \
                 tc.tile_pool(name="vtp", bufs=1, space="PSUM") as vtp:
                for sc in range(32):
                    r = sc % 3
                    pf = psF.tile([128, 512], F32, tag="pf")
                    if sc < 4:
                        nc.tensor.matmul(pf[:, 0:256], ones1[:],
                                         brow[:, r * 256:(r + 1) * 256],
                                         start=True, stop=False)
                    for cb in range(2):
                        nc.tensor.matmul(
                            pf[:, 0:256],
                            xsb[cb][:, sc * 128:(sc + 1) * 128],
                            wq_sl(cb, r),
                            start=(cb == 0 and sc >= 4), stop=(cb == 1),
                        )
                    if sc < 4:
                        nc.scalar.copy(QKVv[:, sc, :], pf[:, 0:256])
                    else:
                        nc.vector.tensor_tensor(
                            out=QKVv[:, sc, :], in0=pf[:, 0:256],
                            in1=btile[:, r * 256:(r + 1) * 256], op=AluOpType.add)
                    # kT staging: one wide copy per 8 blocks
                    if sc % 8 == 7:
                        o8 = sc - 7
                        nc.vector.tensor_scalar_add(
                            kT[:, o8 * 64:(o8 + 8) * 64],
                            QKVv[:, o8:o8 + 8, 128:192], 0.0)
                    # v transpose for block jb once its two source blocks are in
                    if sc >= 3 and sc % 2 == 1:
                        jb = (sc - 3) // 2
                        tp = vtp.tile([128, 1024], BF16, name="vtp0", tag="vtp")
                        nc.tensor.transpose(tp[0:64, 0:128], v_src(2 * jb + 1), ident)
                        nc.tensor.transpose(tp[64:128, 0:128], v_src(2 * jb + 2),
                                            ident, tile_position=(0, 64))
                        nc.vector.tensor_scalar_add(v_sl(jb), tp[:, 0:128], 0.0)
                # jb14 (blocks 29,30) and jb15 (blocks 31, 0 - wraps)
                for jb, (b1, b2) in ((14, (29, 30)), (15, (31, 0))):
                    tp = vtp.tile([128, 1024], BF16, name="vtp0", tag="vtp")
                    nc.tensor.transpose(tp[0:64, 0:128], v_src(b1), ident)
                    nc.tensor.transpose(tp[64:128, 0:128], v_src(b2), ident,
                                        tile_position=(0, 64))
                    nc.vector.tensor_scalar_add(v_sl(jb), tp[:, 0:128], 0.0)

            # ---- phase A ----
            # chunk order: lead h-rotation for jb0-3 (chases stage 1), then
            # per-jb h0..h3 so stats arrive early and steadily.
            order = []
            for h in range(4):
                for jb in range(4):
                    order.append((jb, h))
            for jb in range(4, 16):
                for h in range(4):
                    order.append((jb, h))

            with tc.tile_pool(name="psBi", bufs=1, space="PSUM") as psBi:
                bankA = [None] * 4
                bankB = [None] * 4
                pe_q = []   # (ready_chunk_idx, emit_fn): deferred PE MMs

                def flush(cidx, budget=4):
                    n = 0
                    while pe_q and pe_q[0][0] <= cidx and n < budget:
                        pe_q.pop(0)[1]()
                        n += 1

                def stats(jb):
                    nc.vector.reduce_sum(
                        out=zsum[:, jb:jb + 1], in_=zacc[:, jb * 4:(jb + 1) * 4],
                        axis=AX.X)
                    nc.vector.reciprocal(zinv[:, jb:jb + 1], zsum[:, jb:jb + 1])
                    nc.vector.tensor_scalar_mul(v_sl(jb), v_sl(jb), zinv[:, jb:jb + 1])

                def out_mm(bank, g, jb, start, stop):
                    nc.tensor.matmul(
                        bank[:], v_sl(jb),
                        P[:, jb * HW + g * 512: jb * HW + (g + 1) * 512],
                        start=start, stop=stop,
                    )

                with tc.tile_pool(name="psA", bufs=2, space="PSUM") as psA:
                    for cidx, (jb, h) in enumerate(order):
                        pa = psA.tile([128, 1024], F32, name="pa", tag="pa")
                        for n2 in range(2):
                            nc.tensor.matmul(
                                pa[:, n2 * 512:(n2 + 1) * 512],
                                kT[:, jb * 128:(jb + 1) * 128],
                                QKVv[:, 8 * h + 4 * n2: 8 * h + 4 * n2 + 4, 0:128],
                                start=True, stop=True,
                            )
                        psl = P[:, jb * HW + h * 1024: jb * HW + (h + 1) * 1024]
                        zc = zacc[:, jb * 4 + h: jb * 4 + h + 1]
                        # Z per chunk: ACT accum on h1/h3 (h3 is
                        # stats-critical), DVE reduces for h0/h2.
                        if h % 2 == 1:
                            nc.scalar.activation(out=psl, in_=pa[:], func=AF.Exp,
                                                 scale=SCALE, accum_out=zc)
                        else:
                            nc.scalar.activation(out=psl, in_=pa[:], func=AF.Exp,
                                                 scale=SCALE)
                            nc.vector.reduce_sum(out=zc, in_=psl, axis=AX.X)
                        flush(cidx)
                        if h == 3:
                            stats(jb)
                            if jb == 0:
                                for g in range(4):
                                    bankA[g] = psBi.tile(
                                        [128, 512], F32, name=f"bk{g}",
                                        tag=f"bk{g}")
                            if jb < 8:
                                for g in range(4):
                                    pe_q.append((cidx + 2,
                                                 (lambda g=g, jb=jb: out_mm(
                                                     bankA[g], g, jb,
                                                     jb == 0, jb == 7))))
                            else:
                                for g in range(4):
                                    pe_q.append((cidx + 2,
                                                 (lambda g=g, jb=jb: out_mm(
                                                     bankB[g], g + 4, jb,
                                                     False, jb == 15))))
                            if jb == 7:
                                # spill A banks; queue B bursts (groups 4-7).
                                # Emitted via pe_q AFTER the A1 jb7 MMs so the
                                # spill orders behind the full chain.
                                def spill_and_b():
                                    for g in range(4):
                                        nc.vector.tensor_copy(spill[g][:],
                                                              bankA[g][:])
                                    for g in range(4):
                                        bankB[g] = psBi.tile(
                                            [128, 512], F32, name=f"bk{g}",
                                            tag=f"bk{g}")
                                pe_q.append((cidx + 2, spill_and_b))
                                for jbq in range(8):
                                    for g in range(4):
                                        pe_q.append((cidx + 2 + jbq // 2,
                                                     (lambda g=g, jbq=jbq: out_mm(
                                                         bankB[g], g + 4, jbq,
                                                         jbq == 0, False))))
                    while pe_q:
                        pe_q.pop(0)[1]()

                # psA closed: B drains on ACT, then A2 on psA's banks
                for g in range(4):
                    nc.scalar.copy(outTa[:, (g + 4) * 512:(g + 5) * 512],
                                   bankB[g][:])
                with tc.tile_pool(name="psA2", bufs=1, space="PSUM") as psA2:
                    for g in range(4):
                        bank = psA2.tile([128, 512], F32, name=f"bA2{g}",
                                         tag=f"bA2{g}")
                        for jb in range(8, 16):
                            out_mm(bank, g, jb, jb == 8, jb == 15)
                        nc.vector.tensor_tensor(
                            out=outTa[:, g * 512:(g + 1) * 512],
                            in0=bank[:], in1=spill[g][:], op=AluOpType.add)

            # ---- tail: transposes -> out2a, proj2 with psi_q-folded APs ----
            out2a3 = out2a.rearrange("p (b t) -> p b t", t=128)
            with tc.tile_pool(name="psC", bufs=4, space="PSUM") as psC, \
                 tc.tile_pool(name="psY", bufs=4, space="PSUM") as psY, \
                 tc.tile_pool(name="late", bufs=2) as late:
                tp_order = [4, 5, 6, 7, 0, 1, 2, 3]
                tp_done = {g: i for i, g in enumerate(tp_order)}
                g_ready = {}
                for G in range(8):
                    need = max(tp_done[sc // 4] for r in _proj2_runs(G) for sc in r)
                    g_ready.setdefault(need, []).append(G)

                def proj2(G):
                    runs = _proj2_runs(G)
                    for cb in range(2):
                        py = psY.tile([128, 512], F32, name="py", tag="py")
                        off = 0
                        for ri, run in enumerate(runs):
                            w = 128 * len(run)
                            rhs = out2a3[:, run[0]:run[-1] + 1:3, :]
                            nc.tensor.matmul(
                                py[:, off:off + w],
                                woutT[:, cb * 128:(cb + 1) * 128], rhs,
                                start=(ri == 0), stop=(ri == len(runs) - 1),
                                skip_group_check=True,
                            )
                            off += w
                        yg = late.tile([128, 512], BF16,
                                       name=f"yg{(G * 2 + cb) % 4}",
                                       tag=f"yg{(G * 2 + cb) % 4}")
                        if cb == 0 or G >= 6:
                            nc.scalar.activation(yg[:], py[:], AF.Identity,
                                                 bias=bo[cb])
                        else:
                            nc.vector.tensor_scalar_add(yg[:], py[:], bo[cb])
                        [nc.sync, nc.scalar][(G + cb) % 2].dma_start(
                            out=y_ext[cb * 128:(cb + 1) * 128,
                                      G * 512:(G + 1) * 512],
                            in_=yg[:])

                for idx, g in enumerate(tp_order):
                    tpc = psC.tile([128, 512], BF16, name="tpc", tag="tpc")
                    for s in range(4):
                        nc.tensor.transpose(
                            tpc[:, s * 128:(s + 1) * 128],
                            outTa[:, g * 512 + s * 128: g * 512 + (s + 1) * 128],
                            ident)
                    if g >= 4:
                        nc.scalar.copy(out2a[:, g * 512:(g + 1) * 512], tpc[:])
                    else:
                        nc.vector.tensor_scalar_add(
                            out2a[:, g * 512:(g + 1) * 512], tpc[:], 0.0)
                    for G in g_ready.get(idx, []):
                        proj2(G)

    nc.compile()
    return nc


def get_nc():
    if "nc" not in _CACHE:
        _CACHE["nc"] = build_nc()
    return _CACHE["nc"]


def make_in_maps(x, W_qkv, b_qkv, W_out, b_out):
    x = np.asarray(x, dtype=np.float32)
    W_qkv = np.asarray(W_qkv, dtype=np.float32)
    b_qkv = np.asarray(b_qkv, dtype=np.float32)
    W_out = np.asarray(W_out, dtype=np.float32)
    b_out = np.asarray(b_out, dtype=np.float32)

    operm = (np.arange(O) + O // 2) % O      # rotate qkv channels by 192
    eperm = (np.arange(E) + E // 2) % E      # rotate e-axis by 64

    halves = []
    for h in range(2):
        if h == 0:
            wq, bqv, wo, bov = W_qkv, b_qkv, W_out, b_out
        else:
            wq = W_qkv[operm]
            bqv = b_qkv[operm]
            wo = W_out[:, eperm]
            bov = np.zeros_like(b_out)
        orders = [
            [3 * t + r for t in range(128)]
            + [3 * t + (r + 2) % 3 for t in range(64)]
            + [3 * t + (r + 1) % 3 for t in range(64)]
            for r in range(3)
        ]
        wqv = np.concatenate([wq.T[:, o] for o in orders], axis=1)     # (C, 768)
        btv = np.concatenate(
            [np.broadcast_to(bqv[o][None, :], (128, 256)) for o in orders], axis=1)
        halves.append({
            "wqkvT": np.ascontiguousarray(wqv).astype(ml_dtypes.bfloat16),
            "btile": np.ascontiguousarray(btv).astype(ml_dtypes.bfloat16),
            "brow": np.ascontiguousarray(btv[0:1, :]).astype(ml_dtypes.bfloat16),
            "woutT": np.ascontiguousarray(wo.T).astype(ml_dtypes.bfloat16),
            "bout": np.ascontiguousarray(bov.reshape(C, 1)),
        })

    xb = [np.ascontiguousarray(x[n].reshape(C, HW)).astype(ml_dtypes.bfloat16)
          for n in range(N)]
    in_maps = []
    for core in range(8):
        n, h = core // 2, core % 2
        m = {"x": xb[n]}
        m.update(halves[h])
        in_maps.append(m)
    return in_maps


def run(inputs, trace=False, **kw):
    nc = get_nc()
    in_maps = make_in_maps(**inputs)
    res = run_bass_kernel_spmd(nc, in_maps, core_ids=list(range(8)), trace=trace, **kw)
    ys = [np.asarray(res.results[i]["out"], dtype=np.float32) for i in range(8)]
    y = np.stack([ys[2 * n] + ys[2 * n + 1] for n in range(N)])
    return y.reshape(N, C, 64, 64), res


def kernel(**inputs):
    y, _ = run(inputs, trace=False)
    return y


# revision 35
# speedup vs baseline: 1.2485x; 1.0270x over previous
"""Trainium2 Bass kernel for nn_Attention (dense_transformer), v2.

Reference computation (per batch n of 4):
  qkv = W_qkv @ x + b          (384, 4096)   [x flattened to (256, 64*64)]
  raw C-order reinterpret of qkv flat buffer as (4096, 384) -> q|k|v (4096,128)
  scores = q @ k.T / 64        (4096, 4096)
  soft = softmax(scores, axis=-2)             [column softmax]
  out = soft @ v               (4096, 128)
  raw reinterpret of out as (128, 4096)
  y = W_out @ out2 + b_out     (256, 4096)

Sharding: 8 cores = 4 batches x 2 j-halves (t-halves of the permuted j
enumeration; the host-side 192-rotation of qkv channels and 64-rotation of
W_out's e-axis make the SPMD program identical on all cores). Host sums the
per-pair partial y.

Dataflow (all on-chip; no DRAM roundtrip):
  The raw reinterpret has block structure: with flat = 128*u + r,
  u = 3*tok + (qkv sel) = 32*o + sc, the d-axis of q/k/v equals r = s%128.
  Stage 1 computes, per 128-col s-block sc, FT = x_blk^T @ W_var[sc%3] in
  PSUM, where the host pre-permutes W's output channels per residue so each
  block lands as contiguous [q(128) | k-half(64) | v-half(64)] (the other
  t-half belongs to the paired core and is never computed). One 256-wide
  copy per block drains PSUM->SBUF: ACT plain copies for sc<4 (bias added
  in PSUM via a ones-row matmul against a bias-row input), DVE
  tensor-tensor adds against a host-broadcast bias tile for the rest.
  kT is restaged contiguously (4 wide DVE copies); v is untransposed with
  2x64-wide TensorE transposes (tile_position for the upper half).

  Phase A: per (jb, h-quarter): 2 score MMs (128j x 1024i) -> PSUM(2 banks,
  double buffered), ACT exp (1024-wide) -> P bf16 (persistent SBUF).
  Z per chunk: ACT accum_out on h1/h3 (h3 gates the per-jb stats), DVE
  reduces for h0/h2. Per-jb stats: Z sum, reciprocal, v-block scale (DVE).
  Output accumulation (8 groups of 512 i' = sc-quads) rotates 4 PSUM banks:
  groups 0-3 chain jb0-7 inline, spill to SBUF f32, groups 4-7 then chain
  jb0-15 (burst jb0-7 from persistent P + live jb8-15); groups 0-3 finish
  jb8-15 in the tail on psA's freed banks, their drain adding the spill.
  All non-score PE work is emitted via a lag-2 side queue (<=4 per chunk)
  so nothing with unmet deps ever blocks the in-order PE stream ahead of
  the next score MMs.

  Tail: per group: PSUM -> outTa bf16, 4 TensorE transposes -> out2a in
  accumulator (sc) order. The psi_q permutation folds into proj2's rhs
  access patterns (stride-3 chunk gathers; y-groups 2 and 5 split into two
  accumulating MMs). y = woutT-mm + bias (ACT/DVE), DMA per (group, chalf).
"""

import numpy as np
import ml_dtypes

import concourse.bass as bass
import concourse.bacc as bacc
import concourse.mybir as mybir
from concourse.bass_utils import run_bass_kernel_spmd
from concourse.tile import TileContext, add_dep_helper
from concourse.masks import make_identity
from concourse.alu_op_type import AluOpType

BF16 = mybir.dt.bfloat16
F32 = mybir.dt.float32
AF = mybir.ActivationFunctionType
AX = mybir.AxisListType

N, C, E, O, HW = 4, 256, 128, 384, 4096
JC = HW // 2          # j per core
NJB = JC // 128       # 16 j-blocks
SCALE = 1.0 / 64.0    # 1/sqrt(HW)

_CACHE = {}


def _psiq_inv(m):
    if m <= 10:
        return 3 * m
    if m <= 21:
        return 3 * (m - 11) + 1
    return 3 * (m - 22) + 2


def _proj2_runs(G):
    """Maximal stride-3 source-chunk runs feeding y columns [4G*128,(4G+4)*128)."""
    srcs = [_psiq_inv(4 * G + k) for k in range(4)]
    runs = []
    for s in srcs:
        if runs and s == runs[-1][-1] + 3:
            runs[-1].append(s)
        else:
            runs.append([s])
    return runs


def build_nc():
    nc = bacc.Bacc("TRN2", target_bir_lowering=False, debug=False, num_devices=8)

    x_ext = nc.dram_tensor("x", [C, HW], BF16, kind="ExternalInput").ap()
    wqkvT_ext = nc.dram_tensor("wqkvT", [C, 768], BF16, kind="ExternalInput").ap()
    btile_ext = nc.dram_tensor("btile", [128, 768], BF16, kind="ExternalInput").ap()
    brow_ext = nc.dram_tensor("brow", [1, 768], BF16, kind="ExternalInput").ap()
    woutT_ext = nc.dram_tensor("woutT", [E, C], BF16, kind="ExternalInput").ap()
    bout_ext = nc.dram_tensor("bout", [C, 1], F32, kind="ExternalInput").ap()
    y_ext = nc.dram_tensor("out", [C, HW], BF16, kind="ExternalOutput").ap()

    # persistent SBUF
    xsb = [nc.alloc_sbuf_tensor(f"x{cb}", [128, HW], BF16).ap() for cb in range(2)]
    QKV = nc.alloc_sbuf_tensor("QKV", [128, 32 * 256], BF16).ap()
    kT = nc.alloc_sbuf_tensor("kT", [128, JC], BF16).ap()
    vsb = nc.alloc_sbuf_tensor("vsb", [128, JC], BF16).ap()
    P = nc.alloc_sbuf_tensor("P", [128, NJB * HW], BF16).ap()
    outTa = nc.alloc_sbuf_tensor("outTa", [128, HW], BF16).ap()
    out2a = nc.alloc_sbuf_tensor("out2a", [128, HW], BF16).ap()
    spill = [nc.alloc_sbuf_tensor(f"spill{g}", [128, 512], F32).ap() for g in range(4)]
    zacc = nc.alloc_sbuf_tensor("zacc", [128, 64], F32).ap()
    zsum = nc.alloc_sbuf_tensor("zsum", [128, 16], F32).ap()
    zinv = nc.alloc_sbuf_tensor("zinv", [128, 16], F32).ap()

    def v_sl(jb):
        return vsb[:, jb * 128:(jb + 1) * 128]

    with TileContext(nc) as tc:
        with tc.tile_pool(name="consts", bufs=1) as consts:
            # ---- weights/constants ----
            brow = consts.tile([1, 768], BF16, name="brow", tag="brow")
            nc.sync.dma_start(out=brow, in_=brow_ext[:])
            wq_all = consts.tile([128, 2 * 768], BF16, name="wq_all", tag="wq_all")
            for cb in range(2):
                nc.scalar.dma_start(out=wq_all[:, cb * 768:(cb + 1) * 768],
                                    in_=wqkvT_ext[cb * 128:(cb + 1) * 128, :])
            btile = consts.tile([128, 768], BF16, name="btile", tag="btile")
            nc.gpsimd.dma_start(out=btile, in_=btile_ext[:])
            ones1 = consts.tile([1, 128], BF16, name="ones1", tag="ones1")
            nc.vector.memset(ones1[:], 1.0)

            def wq_sl(cb, r):
                return wq_all[:, cb * 768 + r * 256: cb * 768 + (r + 1) * 256]

            misc = consts.tile([128, C + 128], BF16, name="misc", tag="misc")
            woutT = misc[:, 0:C]
            ident = misc[:, C:C + 128]
            nc.gpsimd.dma_start(out=woutT, in_=woutT_ext[:])
            make_identity(nc, ident)
            bo2 = consts.tile([128, 2], F32, name="bo2", tag="bo2")
            bo = [bo2[:, cb:cb + 1] for cb in range(2)]
            for cb in range(2):
                nc.gpsimd.dma_start(out=bo[cb], in_=bout_ext[cb * 128:(cb + 1) * 128, :])
            # Exp table preload
            scratch = consts.tile([128, 1], F32, name="scratch", tag="scratch")
            nc.vector.memset(scratch[:], 0.0)
            nc.scalar.activation(scratch[:], scratch[:], AF.Exp)

            # ---- PE warmup (p-state ramp) ----
            wsrc = consts.tile([128, 128], BF16, name="wsrc", tag="wsrc")
            nc.vector.memset(wsrc[:], 1.0)
            with tc.tile_pool(name="psW", bufs=1, space="PSUM") as psW:
                wtile = psW.tile([128, 128], F32, tag="warm")
                for _ in range(16):
                    nc.tensor.matmul(wtile[:], wsrc[:], wsrc[:], start=True, stop=True)

            # ---- x loads: 8 quarter-chunks on sync/scalar rings ----
            for qtr in range(4):
                for cb in range(2):
                    eng = nc.sync if (qtr + cb) % 2 == 0 else nc.scalar
                    eng.dma_start(
                        out=xsb[cb][:, qtr * 1024:(qtr + 1) * 1024],
                        in_=x_ext[cb * 128:(cb + 1) * 128,
                                  qtr * 1024:(qtr + 1) * 1024])

            # ---- stage 1: FT blocks (variant-permuted W) -> QKV + v transposes ----
            QKVv = QKV.rearrange("p (b c) -> p b c", c=256)

            def v_src(b):
                return QKVv[:, b, 192:256]

            with tc.tile_pool(name="psF", bufs=3, space="PSUM") as psF, \
                 tc.tile_pool(name="vtp", bufs=1, space="PSUM") as vtp:
                for sc in range(32):
                    r = sc % 3
                    pf = psF.tile([128, 512], F32, tag="pf")
                    if sc < 4:
                        nc.tensor.matmul(pf[:, 0:256], ones1[:],
                                         brow[:, r * 256:(r + 1) * 256],
                                         start=True, stop=False)
                    for cb in range(2):
                        nc.tensor.matmul(
                            pf[:, 0:256],
                            xsb[cb][:, sc * 128:(sc + 1) * 128],
                            wq_sl(cb, r),
                            start=(cb == 0 and sc >= 4), stop=(cb == 1),
                        )
                    if sc < 4:
                        nc.scalar.copy(QKVv[:, sc, :], pf[:, 0:256])
                    else:
                        nc.vector.tensor_tensor(
                            out=QKVv[:, sc, :], in0=pf[:, 0:256],
                            in1=btile[:, r * 256:(r + 1) * 256], op=AluOpType.add)
                    # kT staging: one wide copy per 8 blocks
                    if sc % 8 == 7:
                        o8 = sc - 7
                        nc.vector.tensor_scalar_add(
                            kT[:, o8 * 64:(o8 + 8) * 64],
                            QKVv[:, o8:o8 + 8, 128:192], 0.0)
                    # v transpose for block jb once its two source blocks are in
                    if sc >= 3 and sc % 2 == 1:
                        jb = (sc - 3) // 2
                        tp = vtp.tile([128, 1024], BF16, name="vtp0", tag="vtp")
                        nc.tensor.transpose(tp[0:64, 0:128], v_src(2 * jb + 1), ident)
                        nc.tensor.transpose(tp[64:128, 0:128], v_src(2 * jb + 2),
                                            ident, tile_position=(0, 64))
                        nc.vector.tensor_scalar_add(v_sl(jb), tp[:, 0:128], 0.0)
                # jb14 (blocks 29,30) and jb15 (blocks 31, 0 - wraps)
                for jb, (b1, b2) in ((14, (29, 30)), (15, (31, 0))):
                    tp = vtp.tile([128, 1024], BF16, name="vtp0", tag="vtp")
                    nc.tensor.transpose(tp[0:64, 0:128], v_src(b1), ident)
                    nc.tensor.transpose(tp[64:128, 0:128], v_src(b2), ident,
                                        tile_position=(0, 64))
                    nc.vector.tensor_scalar_add(v_sl(jb), tp[:, 0:128], 0.0)

            # ---- phase A ----
            # chunk order: lead h-rotation for jb0-3 (chases stage 1), then
            # per-jb h0..h3 so stats arrive early and steadily.
            order = []
            for h in range(4):
                for jb in range(4):
                    order.append((jb, h))
            for jb in range(4, 16):
                for h in range(4):
                    order.append((jb, h))

            with tc.tile_pool(name="psBi", bufs=1, space="PSUM") as psBi:
                bankA = [None] * 4
                bankB = [None] * 4
                pe_q = []   # (ready_chunk_idx, emit_fn): deferred PE MMs

                def flush(cidx, budget=1):
                    n = 0
                    while pe_q and pe_q[0][0] <= cidx and n < budget:
                        pe_q.pop(0)[1]()
                        n += 1

                def stats(jb):
                    nc.vector.reduce_sum(
                        out=zsum[:, jb:jb + 1], in_=zacc[:, jb * 4:(jb + 1) * 4],
                        axis=AX.X)
                    nc.vector.reciprocal(zinv[:, jb:jb + 1], zsum[:, jb:jb + 1])
                    nc.vector.tensor_scalar_mul(v_sl(jb), v_sl(jb), zinv[:, jb:jb + 1])

                def out_mm(bank, g, jb, start, stop):
                    nc.tensor.matmul(
                        bank[:], v_sl(jb),
                        P[:, jb * HW + g * 512: jb * HW + (g + 1) * 512],
                        start=start, stop=stop,
                    )

                with tc.tile_pool(name="psA", bufs=2, space="PSUM") as psA:
                    for cidx, (jb, h) in enumerate(order):
                        pa = psA.tile([128, 1024], F32, name="pa", tag="pa")
                        for n2 in range(2):
                            nc.tensor.matmul(
                                pa[:, n2 * 512:(n2 + 1) * 512],
                                kT[:, jb * 128:(jb + 1) * 128],
                                QKVv[:, 8 * h + 4 * n2: 8 * h + 4 * n2 + 4, 0:128],
                                start=True, stop=True,
                            )
                        psl = P[:, jb * HW + h * 1024: jb * HW + (h + 1) * 1024]
                        zc = zacc[:, jb * 4 + h: jb * 4 + h + 1]
                        # Z per chunk: ACT accum on h1/h3 (h3 is
                        # stats-critical), DVE reduces for h0/h2.
                        if h % 2 == 1:
                            nc.scalar.activation(out=psl, in_=pa[:], func=AF.Exp,
                                                 scale=SCALE, accum_out=zc)
                        else:
                            nc.scalar.activation(out=psl, in_=pa[:], func=AF.Exp,
                                                 scale=SCALE)
                            nc.vector.reduce_sum(out=zc, in_=psl, axis=AX.X)
                        flush(cidx)
                        if h == 3:
                            stats(jb)
                            if jb == 0:
                                for g in range(4):
                                    bankA[g] = psBi.tile(
                                        [128, 512], F32, name=f"bk{g}",
                                        tag=f"bk{g}")
                            if jb < 8:
                                for g in range(4):
                                    pe_q.append((cidx + 2,
                                                 (lambda g=g, jb=jb: out_mm(
                                                     bankA[g], g, jb,
                                                     jb == 0, jb == 7))))
                            else:
                                for g in range(4):
                                    pe_q.append((cidx + 2,
                                                 (lambda g=g, jb=jb: out_mm(
                                                     bankB[g], g + 4, jb,
                                                     False, jb == 15))))
                            if jb == 7:
                                # spill A banks; queue B bursts (groups 4-7).
                                # Emitted via pe_q AFTER the A1 jb7 MMs so the
                                # spill orders behind the full chain.
                                def spill_and_b():
                                    for g in range(4):
                                        nc.vector.tensor_copy(spill[g][:],
                                                              bankA[g][:])
                                    for g in range(4):
                                        bankB[g] = psBi.tile(
                                            [128, 512], F32, name=f"bk{g}",
                                            tag=f"bk{g}")
                                pe_q.append((cidx + 2, spill_and_b))
                                for jbq in range(8):
                                    for g in range(4):
                                        pe_q.append((cidx + 2 + jbq // 2,
                                                     (lambda g=g, jbq=jbq: out_mm(
                                                         bankB[g], g + 4, jbq,
                                                         jbq == 0, False))))
                    while pe_q:
                        pe_q.pop(0)[1]()

                # psA closed: B drains on ACT, then A2 on psA's banks
                for g in range(4):
                    nc.scalar.copy(outTa[:, (g + 4) * 512:(g + 5) * 512],
                                   bankB[g][:])
                with tc.tile_pool(name="psA2", bufs=1, space="PSUM") as psA2:
                    for g in range(4):
                        bank = psA2.tile([128, 512], F32, name=f"bA2{g}",
                                         tag=f"bA2{g}")
                        for jb in range(8, 16):
                            out_mm(bank, g, jb, jb == 8, jb == 15)
                        nc.vector.tensor_tensor(
                            out=outTa[:, g * 512:(g + 1) * 512],
                            in0=bank[:], in1=spill[g][:], op=AluOpType.add)

            # ---- tail: transposes -> out2a, proj2 with psi_q-folded APs ----
            out2a3 = out2a.rearrange("p (b t) -> p b t", t=128)
            with tc.tile_pool(name="psC", bufs=4, space="PSUM") as psC, \
                 tc.tile_pool(name="psY", bufs=4, space="PSUM") as psY, \
                 tc.tile_pool(name="late", bufs=2) as late:
                tp_order = [4, 5, 6, 7, 0, 1, 2, 3]
                tp_done = {g: i for i, g in enumerate(tp_order)}
                g_ready = {}
                for G in range(8):
                    need = max(tp_done[sc // 4] for r in _proj2_runs(G) for sc in r)
                    g_ready.setdefault(need, []).append(G)

                def proj2(G):
                    runs = _proj2_runs(G)
                    for cb in range(2):
                        py = psY.tile([128, 512], F32, name="py", tag="py")
                        off = 0
                        for ri, run in enumerate(runs):
                            w = 128 * len(run)
                            rhs = out2a3[:, run[0]:run[-1] + 1:3, :]
                            nc.tensor.matmul(
                                py[:, off:off + w],
                                woutT[:, cb * 128:(cb + 1) * 128], rhs,
                                start=(ri == 0), stop=(ri == len(runs) - 1),
                                skip_group_check=True,
                            )
                            off += w
                        yg = late.tile([128, 512], BF16,
                                       name=f"yg{(G * 2 + cb) % 4}",
                                       tag=f"yg{(G * 2 + cb) % 4}")
                        if cb == 0 or G >= 6:
                            nc.scalar.activation(yg[:], py[:], AF.Identity,
                                                 bias=bo[cb])
                        else:
                            nc.vector.tensor_scalar_add(yg[:], py[:], bo[cb])
                        [nc.sync, nc.scalar][(G + cb) % 2].dma_start(
                            out=y_ext[cb * 128:(cb + 1) * 128,
                                      G * 512:(G + 1) * 512],
                            in_=yg[:])

                for idx, g in enumerate(tp_order):
                    tpc = psC.tile([128, 512], BF16, name="tpc", tag="tpc")
                    for s in range(4):
                        nc.tensor.transpose(
                            tpc[:, s * 128:(s + 1) * 128],
                            outTa[:, g * 512 + s * 128: g * 512 + (s + 1) * 128],
                            ident)
                    if g >= 4:
                        nc.scalar.copy(out2a[:, g * 512:(g + 1) * 512], tpc[:])
                    else:
                        nc.vector.tensor_scalar_add(
                            out2a[:, g * 512:(g + 1) * 512], tpc[:], 0.0)
                    for G in g_ready.get(idx, []):
                        proj2(G)

    nc.compile()
    return nc


def get_nc():
    if "nc" not in _CACHE:
        _CACHE["nc"] = build_nc()
    return _CACHE["nc"]


def make_in_maps(x, W_qkv, b_qkv, W_out, b_out):
    x = np.asarray(x, dtype=np.float32)
    W_qkv = np.asarray(W_qkv, dtype=np.float32)
    b_qkv = np.asarray(b_qkv, dtype=np.float32)
    W_out = np.asarray(W_out, dtype=np.float32)
    b_out = np.asarray(b_out, dtype=np.float32)

    operm = (np.arange(O) + O // 2) % O      # rotate qkv channels by 192
    eperm = (np.arange(E) + E // 2) % E      # rotate e-axis by 64

    halves = []
    for h in range(2):
        if h == 0:
            wq, bqv, wo, bov = W_qkv, b_qkv, W_out, b_out
        else:
            wq = W_qkv[operm]
            bqv = b_qkv[operm]
            wo = W_out[:, eperm]
            bov = np.zeros_like(b_out)
        orders = [
            [3 * t + r for t in range(128)]
            + [3 * t + (r + 2) % 3 for t in range(64)]
            + [3 * t + (r + 1) % 3 for t in range(64)]
            for r in range(3)
        ]
        wqv = np.concatenate([wq.T[:, o] for o in orders], axis=1)     # (C, 768)
        btv = np.concatenate(
            [np.broadcast_to(bqv[o][None, :], (128, 256)) for o in orders], axis=1)
        halves.append({
            "wqkvT": np.ascontiguousarray(wqv).astype(ml_dtypes.bfloat16),
            "btile": np.ascontiguousarray(btv).astype(ml_dtypes.bfloat16),
            "brow": np.ascontiguousarray(btv[0:1, :]).astype(ml_dtypes.bfloat16),
            "woutT": np.ascontiguousarray(wo.T).astype(ml_dtypes.bfloat16),
            "bout": np.ascontiguousarray(bov.reshape(C, 1)),
        })

    xb = [np.ascontiguousarray(x[n].reshape(C, HW)).astype(ml_dtypes.bfloat16)
          for n in range(N)]
    in_maps = []
    for core in range(8):
        n, h = core // 2, core % 2
        m = {"x": xb[n]}
        m.update(halves[h])
        in_maps.append(m)
    return in_maps


def run(inputs, trace=False, **kw):
    nc = get_nc()
    in_maps = make_in_maps(**inputs)
    res = run_bass_kernel_spmd(nc, in_maps, core_ids=list(range(8)), trace=trace, **kw)
    ys = [np.asarray(res.results[i]["out"], dtype=np.float32) for i in range(8)]
    y = np.stack([ys[2 * n] + ys[2 * n + 1] for n in range(N)])
    return y.reshape(N, C, 64, 64), res


def kernel(**inputs):
    y, _ = run(inputs, trace=False)
    return y


# revision 38
# speedup vs baseline: 1.2583x; 1.0078x over previous
"""Trainium2 Bass kernel for nn_Attention (dense_transformer), v2.

Reference computation (per batch n of 4):
  qkv = W_qkv @ x + b          (384, 4096)   [x flattened to (256, 64*64)]
  raw C-order reinterpret of qkv flat buffer as (4096, 384) -> q|k|v (4096,128)
  scores = q @ k.T / 64        (4096, 4096)
  soft = softmax(scores, axis=-2)             [column softmax]
  out = soft @ v               (4096, 128)
  raw reinterpret of out as (128, 4096)
  y = W_out @ out2 + b_out     (256, 4096)

Sharding: 8 cores = 4 batches x 2 j-halves (t-halves of the permuted j
enumeration; the host-side 192-rotation of qkv channels and 64-rotation of
W_out's e-axis make the SPMD program identical on all cores). Host sums the
per-pair partial y.

Dataflow (all on-chip; no DRAM roundtrip):
  The raw reinterpret has block structure: with flat = 128*u + r,
  u = 3*tok + (qkv sel) = 32*o + sc, the d-axis of q/k/v equals r = s%128.
  Stage 1 computes, per 128-col s-block sc, FT = x_blk^T @ W_var[sc%3] in
  PSUM, where the host pre-permutes W's output channels per residue so each
  block lands as contiguous [q(128) | k-half(64) | v-half(64)] (the other
  t-half belongs to the paired core and is never computed). One 256-wide
  copy per block drains PSUM->SBUF: ACT plain copies for sc<4 (bias added
  in PSUM via a ones-row matmul against a bias-row input), DVE
  tensor-tensor adds against a host-broadcast bias tile for the rest.
  kT is restaged contiguously (4 wide DVE copies); v is untransposed with
  2x64-wide TensorE transposes (tile_position for the upper half).

  Phase A: per (jb, h-quarter): 2 score MMs (128j x 1024i) -> PSUM(2 banks,
  double buffered), ACT exp (1024-wide) -> P bf16 (persistent SBUF).
  Z per chunk: ACT accum_out on h1/h3 (h3 gates the per-jb stats), DVE
  reduces for h0/h2. Per-jb stats: Z sum, reciprocal, v-block scale (DVE).
  Output accumulation (8 groups of 512 i' = sc-quads) rotates 4 PSUM banks:
  groups 0-3 chain jb0-7 inline, spill to SBUF f32, groups 4-7 then chain
  jb0-15 (burst jb0-7 from persistent P + live jb8-15); groups 0-3 finish
  jb8-15 in the tail on psA's freed banks, their drain adding the spill.
  All non-score PE work is emitted via a lag-2 side queue (<=4 per chunk)
  so nothing with unmet deps ever blocks the in-order PE stream ahead of
  the next score MMs.

  Tail: per group: PSUM -> outTa bf16, 4 TensorE transposes -> out2a in
  accumulator (sc) order. The psi_q permutation folds into proj2's rhs
  access patterns (stride-3 chunk gathers; y-groups 2 and 5 split into two
  accumulating MMs). y = woutT-mm + bias (ACT/DVE), DMA per (group, chalf).
"""

import numpy as np
import ml_dtypes

import concourse.bass as bass
import concourse.bacc as bacc
import concourse.mybir as mybir
from concourse.bass_utils import run_bass_kernel_spmd
from concourse.tile import TileContext, add_dep_helper
from concourse.masks import make_identity
from concourse.alu_op_type import AluOpType

BF16 = mybir.dt.bfloat16
F32 = mybir.dt.float32
AF = mybir.ActivationFunctionType
AX = mybir.AxisListType

N, C, E, O, HW = 4, 256, 128, 384, 4096
JC = HW // 2          # j per core
NJB = JC // 128       # 16 j-blocks
SCALE = 1.0 / 64.0    # 1/sqrt(HW)

_CACHE = {}


def _psiq_inv(m):
    if m <= 10:
        return 3 * m
    if m <= 21:
        return 3 * (m - 11) + 1
    return 3 * (m - 22) + 2


def _proj2_runs(G):
    """Maximal stride-3 source-chunk runs feeding y columns [4G*128,(4G+4)*128)."""
    srcs = [_psiq_inv(4 * G + k) for k in range(4)]
    runs = []
    for s in srcs:
        if runs and s == runs[-1][-1] + 3:
            runs[-1].append(s)
        else:
            runs.append([s])
    return runs


def build_nc():
    nc = bacc.Bacc("TRN2", target_bir_lowering=False, debug=False, num_devices=8)

    x_ext = nc.dram_tensor("x", [C, HW], BF16, kind="ExternalInput").ap()
    wqkvT_ext = nc.dram_tensor("wqkvT", [C, 768], BF16, kind="ExternalInput").ap()
    btile_ext = nc.dram_tensor("btile", [128, 768], BF16, kind="ExternalInput").ap()
    brow_ext = nc.dram_tensor("brow", [1, 768], BF16, kind="ExternalInput").ap()
    woutT_ext = nc.dram_tensor("woutT", [E, C], BF16, kind="ExternalInput").ap()
    bout_ext = nc.dram_tensor("bout", [C, 1], F32, kind="ExternalInput").ap()
    y_ext = nc.dram_tensor("out", [C, HW], BF16, kind="ExternalOutput").ap()

    # persistent SBUF
    xsb = [nc.alloc_sbuf_tensor(f"x{cb}", [128, HW], BF16).ap() for cb in range(2)]
    QKV = nc.alloc_sbuf_tensor("QKV", [128, 32 * 256], BF16).ap()
    kT = nc.alloc_sbuf_tensor("kT", [128, JC], BF16).ap()
    vsb = nc.alloc_sbuf_tensor("vsb", [128, JC], BF16).ap()
    P = nc.alloc_sbuf_tensor("P", [128, NJB * HW], BF16).ap()
    outTa = nc.alloc_sbuf_tensor("outTa", [128, HW], BF16).ap()
    out2a = nc.alloc_sbuf_tensor("out2a", [128, HW], BF16).ap()
    spill = [nc.alloc_sbuf_tensor(f"spill{g}", [128, 512], F32).ap() for g in range(4)]
    zacc = nc.alloc_sbuf_tensor("zacc", [128, 64], F32).ap()
    zsum = nc.alloc_sbuf_tensor("zsum", [128, 16], F32).ap()
    zinv = nc.alloc_sbuf_tensor("zinv", [128, 16], F32).ap()

    def v_sl(jb):
        return vsb[:, jb * 128:(jb + 1) * 128]

    with TileContext(nc) as tc:
        with tc.tile_pool(name="consts", bufs=1) as consts:
            # ---- weights/constants ----
            brow = consts.tile([1, 768], BF16, name="brow", tag="brow")
            nc.sync.dma_start(out=brow, in_=brow_ext[:])
            wq_all = consts.tile([128, 2 * 768], BF16, name="wq_all", tag="wq_all")
            for cb in range(2):
                nc.scalar.dma_start(out=wq_all[:, cb * 768:(cb + 1) * 768],
                                    in_=wqkvT_ext[cb * 128:(cb + 1) * 128, :])
            btile = consts.tile([128, 768], BF16, name="btile", tag="btile")
            nc.gpsimd.dma_start(out=btile, in_=btile_ext[:])
            ones1 = consts.tile([1, 128], BF16, name="ones1", tag="ones1")
            nc.vector.memset(ones1[:], 1.0)

            def wq_sl(cb, r):
                return wq_all[:, cb * 768 + r * 256: cb * 768 + (r + 1) * 256]

            misc = consts.tile([128, C + 128], BF16, name="misc", tag="misc")
            woutT = misc[:, 0:C]
            ident = misc[:, C:C + 128]
            nc.gpsimd.dma_start(out=woutT, in_=woutT_ext[:])
            make_identity(nc, ident)
            bo2 = consts.tile([128, 2], F32, name="bo2", tag="bo2")
            bo = [bo2[:, cb:cb + 1] for cb in range(2)]
            for cb in range(2):
                nc.gpsimd.dma_start(out=bo[cb], in_=bout_ext[cb * 128:(cb + 1) * 128, :])
            # Exp table preload
            scratch = consts.tile([128, 1], F32, name="scratch", tag="scratch")
            nc.vector.memset(scratch[:], 0.0)
            nc.scalar.activation(scratch[:], scratch[:], AF.Exp)

            # ---- PE warmup (p-state ramp) ----
            wsrc = consts.tile([128, 128], BF16, name="wsrc", tag="wsrc")
            nc.vector.memset(wsrc[:], 1.0)
            with tc.tile_pool(name="psW", bufs=1, space="PSUM") as psW:
                wtile = psW.tile([128, 128], F32, tag="warm")
                for _ in range(16):
                    nc.tensor.matmul(wtile[:], wsrc[:], wsrc[:], start=True, stop=True)

            # ---- x loads: 8 quarter-chunks on sync/scalar rings ----
            for qtr in range(4):
                for cb in range(2):
                    eng = nc.sync if (qtr + cb) % 2 == 0 else nc.scalar
                    eng.dma_start(
                        out=xsb[cb][:, qtr * 1024:(qtr + 1) * 1024],
                        in_=x_ext[cb * 128:(cb + 1) * 128,
                                  qtr * 1024:(qtr + 1) * 1024])

            # ---- stage 1: FT blocks (variant-permuted W) -> QKV + v transposes ----
            QKVv = QKV.rearrange("p (b c) -> p b c", c=256)

            def v_src(b):
                return QKVv[:, b, 192:256]

            with tc.tile_pool(name="psF", bufs=3, space="PSUM") as psF, \
                 tc.tile_pool(name="vtp", bufs=1, space="PSUM") as vtp:
                for sc in range(32):
                    r = sc % 3
                    pf = psF.tile([128, 512], F32, tag="pf")
                    if sc < 4:
                        nc.tensor.matmul(pf[:, 0:256], ones1[:],
                                         brow[:, r * 256:(r + 1) * 256],
                                         start=True, stop=False)
                    for cb in range(2):
                        nc.tensor.matmul(
                            pf[:, 0:256],
                            xsb[cb][:, sc * 128:(sc + 1) * 128],
                            wq_sl(cb, r),
                            start=(cb == 0 and sc >= 4), stop=(cb == 1),
                        )
                    if sc < 4:
                        nc.scalar.copy(QKVv[:, sc, :], pf[:, 0:256])
                    else:
                        nc.vector.tensor_tensor(
                            out=QKVv[:, sc, :], in0=pf[:, 0:256],
                            in1=btile[:, r * 256:(r + 1) * 256], op=AluOpType.add)
                    # kT staging: one wide copy per 8 blocks
                    if sc % 8 == 7:
                        o8 = sc - 7
                        nc.vector.tensor_scalar_add(
                            kT[:, o8 * 64:(o8 + 8) * 64],
                            QKVv[:, o8:o8 + 8, 128:192], 0.0)
                    # v transpose for block jb once its two source blocks are in
                    if sc >= 3 and sc % 2 == 1:
                        jb = (sc - 3) // 2
                        tp = vtp.tile([128, 1024], BF16, name="vtp0", tag="vtp")
                        nc.tensor.transpose(tp[0:64, 0:128], v_src(2 * jb + 1), ident)
                        nc.tensor.transpose(tp[64:128, 0:128], v_src(2 * jb + 2),
                                            ident, tile_position=(0, 64))
                        nc.vector.tensor_scalar_add(v_sl(jb), tp[:, 0:128], 0.0)
                # jb14 (blocks 29,30) and jb15 (blocks 31, 0 - wraps)
                for jb, (b1, b2) in ((14, (29, 30)), (15, (31, 0))):
                    tp = vtp.tile([128, 1024], BF16, name="vtp0", tag="vtp")
                    nc.tensor.transpose(tp[0:64, 0:128], v_src(b1), ident)
                    nc.tensor.transpose(tp[64:128, 0:128], v_src(b2), ident,
                                        tile_position=(0, 64))
                    nc.vector.tensor_scalar_add(v_sl(jb), tp[:, 0:128], 0.0)

            # ---- phase A ----
            # chunk order: lead h-rotation for jb0-3 (chases stage 1), then
            # per-jb h0..h3 so stats arrive early and steadily.
            order = []
            for h in range(4):
                for jb in range(4):
                    order.append((jb, h))
            for jb in range(4, 16):
                for h in range(4):
                    order.append((jb, h))

            with tc.tile_pool(name="psBi", bufs=1, space="PSUM") as psBi:
                bankA = [None] * 4
                bankB = [None] * 4
                pe_q = []   # (ready_chunk_idx, emit_fn): deferred PE MMs

                def flush(cidx, budget=1):
                    n = 0
                    while pe_q and pe_q[0][0] <= cidx and n < budget:
                        pe_q.pop(0)[1]()
                        n += 1

                def stats(jb):
                    nc.vector.reduce_sum(
                        out=zsum[:, jb:jb + 1], in_=zacc[:, jb * 4:(jb + 1) * 4],
                        axis=AX.X)
                    nc.vector.reciprocal(zinv[:, jb:jb + 1], zsum[:, jb:jb + 1])
                    nc.vector.tensor_scalar_mul(v_sl(jb), v_sl(jb), zinv[:, jb:jb + 1])

                def out_mm(bank, g, jb, start, stop):
                    nc.tensor.matmul(
                        bank[:], v_sl(jb),
                        P[:, jb * HW + g * 512: jb * HW + (g + 1) * 512],
                        start=start, stop=stop,
                    )

                with tc.tile_pool(name="psA", bufs=2, space="PSUM") as psA:
                    for cidx, (jb, h) in enumerate(order):
                        pa = psA.tile([128, 1024], F32, name="pa", tag="pa")
                        for n2 in range(2):
                            nc.tensor.matmul(
                                pa[:, n2 * 512:(n2 + 1) * 512],
                                kT[:, jb * 128:(jb + 1) * 128],
                                QKVv[:, 8 * h + 4 * n2: 8 * h + 4 * n2 + 4, 0:128],
                                start=True, stop=True,
                            )
                        psl = P[:, jb * HW + h * 1024: jb * HW + (h + 1) * 1024]
                        zc = zacc[:, jb * 4 + h: jb * 4 + h + 1]
                        # Z per chunk: ACT accum on h1/h3 (h3 is
                        # stats-critical), DVE reduces for h0/h2.
                        if h % 2 == 1:
                            nc.scalar.activation(out=psl, in_=pa[:], func=AF.Exp,
                                                 scale=SCALE, accum_out=zc)
                        else:
                            nc.scalar.activation(out=psl, in_=pa[:], func=AF.Exp,
                                                 scale=SCALE)
                            nc.vector.reduce_sum(out=zc, in_=psl, axis=AX.X)
                        flush(cidx)
                        if h == 3:
                            stats(jb)
                            if jb == 0:
                                for g in range(4):
                                    bankA[g] = psBi.tile(
                                        [128, 512], F32, name=f"bk{g}",
                                        tag=f"bk{g}")
                            if jb < 8:
                                for g in range(4):
                                    pe_q.append((cidx + 2,
                                                 (lambda g=g, jb=jb: out_mm(
                                                     bankA[g], g, jb,
                                                     jb == 0, jb == 7))))
                            else:
                                for g in range(4):
                                    pe_q.append((cidx + 2,
                                                 (lambda g=g, jb=jb: out_mm(
                                                     bankB[g], g + 4, jb,
                                                     False, jb == 15))))
                            if jb == 7:
                                # spill A banks; queue B bursts (groups 4-7).
                                # Emitted via pe_q AFTER the A1 jb7 MMs so the
                                # spill orders behind the full chain.
                                def spill_and_b():
                                    for g in range(4):
                                        nc.vector.tensor_copy(spill[g][:],
                                                              bankA[g][:])
                                    for g in range(4):
                                        bankB[g] = psBi.tile(
                                            [128, 512], F32, name=f"bk{g}",
                                            tag=f"bk{g}")
                                pe_q.append((cidx + 2, spill_and_b))
                                for jbq in range(8):
                                    for g in range(4):
                                        pe_q.append((cidx + 2 + jbq // 2,
                                                     (lambda g=g, jbq=jbq: out_mm(
                                                         bankB[g], g + 4, jbq,
                                                         jbq == 0, False))))
                    while pe_q:
                        pe_q.pop(0)[1]()

                # psA closed: B drains on ACT, then A2 on psA's banks
                for g in range(4):
                    if g % 2 == 0:
                        nc.scalar.copy(outTa[:, (g + 4) * 512:(g + 5) * 512],
                                       bankB[g][:])
                    else:
                        nc.vector.tensor_copy(
                            outTa[:, (g + 4) * 512:(g + 5) * 512], bankB[g][:])
                with tc.tile_pool(name="psA2", bufs=1, space="PSUM") as psA2:
                    for g in range(4):
                        bank = psA2.tile([128, 512], F32, name=f"bA2{g}",
                                         tag=f"bA2{g}")
                        for jb in range(8, 16):
                            out_mm(bank, g, jb, jb == 8, jb == 15)
                        nc.vector.tensor_tensor(
                            out=outTa[:, g * 512:(g + 1) * 512],
                            in0=bank[:], in1=spill[g][:], op=AluOpType.add)

            # ---- tail: transposes -> out2a, proj2 with psi_q-folded APs ----
            out2a3 = out2a.rearrange("p (b t) -> p b t", t=128)
            with tc.tile_pool(name="psC", bufs=4, space="PSUM") as psC, \
                 tc.tile_pool(name="psY", bufs=4, space="PSUM") as psY, \
                 tc.tile_pool(name="late", bufs=2) as late:
                tp_order = [4, 5, 6, 7, 0, 1, 2, 3]
                tp_done = {g: i for i, g in enumerate(tp_order)}
                g_ready = {}
                for G in range(8):
                    need = max(tp_done[sc // 4] for r in _proj2_runs(G) for sc in r)
                    g_ready.setdefault(need, []).append(G)

                def proj2(G):
                    runs = _proj2_runs(G)
                    for cb in range(2):
                        py = psY.tile([128, 512], F32, name="py", tag="py")
                        off = 0
                        for ri, run in enumerate(runs):
                            w = 128 * len(run)
                            rhs = out2a3[:, run[0]:run[-1] + 1:3, :]
                            nc.tensor.matmul(
                                py[:, off:off + w],
                                woutT[:, cb * 128:(cb + 1) * 128], rhs,
                                start=(ri == 0), stop=(ri == len(runs) - 1),
                                skip_group_check=True,
                            )
                            off += w
                        yg = late.tile([128, 512], BF16,
                                       name=f"yg{(G * 2 + cb) % 4}",
                                       tag=f"yg{(G * 2 + cb) % 4}")
                        if cb == 0:
                            nc.scalar.activation(yg[:], py[:], AF.Identity,
                                                 bias=bo[cb])
                        else:
                            nc.vector.tensor_scalar_add(yg[:], py[:], bo[cb])
                        [nc.sync, nc.scalar][(G + cb) % 2].dma_start(
                            out=y_ext[cb * 128:(cb + 1) * 128,
                                      G * 512:(G + 1) * 512],
                            in_=yg[:])

                for idx, g in enumerate(tp_order):
                    tpc = psC.tile([128, 512], BF16, name="tpc", tag="tpc")
                    for s in range(4):
                        nc.tensor.transpose(
                            tpc[:, s * 128:(s + 1) * 128],
                            outTa[:, g * 512 + s * 128: g * 512 + (s + 1) * 128],
                            ident)
                    nc.vector.tensor_scalar_add(
                        out2a[:, g * 512:(g + 1) * 512], tpc[:], 0.0)
                    for G in g_ready.get(idx, []):
                        proj2(G)

    nc.compile()
    return nc


def get_nc():
    if "nc" not in _CACHE:
        _CACHE["nc"] = build_nc()
    return _CACHE["nc"]


def make_in_maps(x, W_qkv, b_qkv, W_out, b_out):
    x = np.asarray(x, dtype=np.float32)
    W_qkv = np.asarray(W_qkv, dtype=np.float32)
    b_qkv = np.asarray(b_qkv, dtype=np.float32)
    W_out = np.asarray(W_out, dtype=np.float32)
    b_out = np.asarray(b_out, dtype=np.float32)

    operm = (np.arange(O) + O // 2) % O      # rotate qkv channels by 192
    eperm = (np.arange(E) + E // 2) % E      # rotate e-axis by 64

    halves = []
    for h in range(2):
        if h == 0:
            wq, bqv, wo, bov = W_qkv, b_qkv, W_out, b_out
        else:
            wq = W_qkv[operm]
            bqv = b_qkv[operm]
            wo = W_out[:, eperm]
            bov = np.zeros_like(b_out)
        orders = [
            [3 * t + r for t in range(128)]
            + [3 * t + (r + 2) % 3 for t in range(64)]
            + [3 * t + (r + 1) % 3 for t in range(64)]
            for r in range(3)
        ]
        wqv = np.concatenate([wq.T[:, o] for o in orders], axis=1)     # (C, 768)
        btv = np.concatenate(
            [np.broadcast_to(bqv[o][None, :], (128, 256)) for o in orders], axis=1)
        halves.append({
            "wqkvT": np.ascontiguousarray(wqv).astype(ml_dtypes.bfloat16),
            "btile": np.ascontiguousarray(btv).astype(ml_dtypes.bfloat16),
            "brow": np.ascontiguousarray(btv[0:1, :]).astype(ml_dtypes.bfloat16),
            "woutT": np.ascontiguousarray(wo.T).astype(ml_dtypes.bfloat16),
            "bout": np.ascontiguousarray(bov.reshape(C, 1)),
        })

    xb = [np.ascontiguousarray(x[n].reshape(C, HW)).astype(ml_dtypes.bfloat16)
          for n in range(N)]
    in_maps = []
    for core in range(8):
        n, h = core // 2, core % 2
        m = {"x": xb[n]}
        m.update(halves[h])
        in_maps.append(m)
    return in_maps


def run(inputs, trace=False, **kw):
    nc = get_nc()
    in_maps = make_in_maps(**inputs)
    res = run_bass_kernel_spmd(nc, in_maps, core_ids=list(range(8)), trace=trace, **kw)
    ys = [np.asarray(res.results[i]["out"], dtype=np.float32) for i in range(8)]
    y = np.stack([ys[2 * n] + ys[2 * n + 1] for n in range(N)])
    return y.reshape(N, C, 64, 64), res


def kernel(**inputs):
    y, _ = run(inputs, trace=False)
    return y
